# revision 17
# baseline (speedup 1.0000x reference)
"""Trainium2 Bass kernel for nn_BoxLoss (masked weighted CIoU loss).

Contract: kernel(**inputs) takes the FULL unsharded inputs
  predicts_bbox [128, 33600, 4] f32, targets_bbox [128, 33600, 4] f32,
  valid_masks [128, 33600] bool, box_norm [128, 33600] f32, cls_norm () f32
and returns the FULL scalar output, sharding batch rows across 8 NeuronCores
internally (pure data parallel, per the sharding hint).

Per-core layout: 16 batch rows x 33600 anchors = 537600 elements laid out
[128 partitions, 4200] (partition-major, each partition owns a contiguous
span). Box coords are de-interleaved on host into planar channels so every
device-side access is contiguous.

Math notes (exact reformulation of the reference):
  d_c  = p_c - t_c ;  wb = t2-t0, hb = t3-t1, wa = p2-p0, ha = p3-p1
  iw   = wb - relu(-d2) - relu(d0)       (== min(p2,t2) - max(p0,t0))
  cw   = wb + relu(d2) + relu(-d0)       (== max(p2,t2) - min(p0,t0))
  cent*4 = (d0+d2)^2 + (d1+d3)^2 ;  diag*4 = (2cw)^2 + (2ch)^2
  => cent*0.25/diag = cent4 / diag4
  atan(u)-atan(v) = atan(T), T=(wa*hb - wb*ha)/(ha*hb + wa*wb), via
  |T|<=1 ? atan(T) : sign(T)*pi/2 - atan(1/T), atan by deg-11 minimax poly.
  Non-overlapping pairs give inter=0 -> ciou = -cd-av < 0 -> loss contrib
  is exactly w (the clip), so fp16 intermediates only perturb overlapping
  pairs (small relative coords) when DT_SMALL = float16.
"""

import sys

if "/opt/trn_rl_repo" not in sys.path:
    sys.path.insert(0, "/opt/trn_rl_repo")

import math
import numpy as np

import concourse.bacc as bacc
from concourse import mybir, tile
from concourse import bass_utils
from concourse import dve_ops as dvo
from concourse.dve_spec import (
    Spec, Src0, Src1, C0, C1, C2, Zero, One, AluOp,
    relu, sq, maxx, minn, select, lower, _has_src1,
)
from concourse.dve_uop import DveOpSpec
from operator import add as _op_add

# ------------------------------- config ------------------------------------
B, A = 128, 33600
N_CORES = 8
B_LOC = B // N_CORES                # 16 batch rows per core
E = B_LOC * A                       # 537600 elements per core
P = 128                             # partitions
F = E // P                          # 4200 free elements per partition
R = 1050                            # chunk free size (divides F)
NCH = F // R

F32 = mybir.dt.float32
F16 = mybir.dt.float16
U8 = mybir.dt.uint8

# dtype of the "small" intermediate chain. float32 is the safe default;
# float16 doubles stock DVE tensor_tensor throughput.
DT_SMALL = F16

HALF_PI = math.pi / 2.0
# minimax atan(x) ~ x*(c0 + c1 z + ... + c5 z^5), z=x^2, |x|<=1, err 1.7e-6
ATAN_C = [0.9999772562021794, -0.3326237246324494, 0.19354622050707823,
          -0.11644164122245204, 0.05266424416536723, -0.011725888127135233]

# --------------------------- custom DVE ops --------------------------------
_my_ops = {}


def _register(name, spec, subdim=False):
    if name in _my_ops:
        return _my_ops[name]
    existing = {op.name: op for op in dvo.OPS}
    if name in existing:
        _my_ops[name] = existing[name]
        return existing[name]
    opcode = dvo._CUSTOM_DVE_ROW_BASE + len(dvo.OPS)
    shas = {}
    for ver in ("v3", "v4"):
        tmp = DveOpSpec(name=name, opcode=opcode, uops=lower(spec, ver=ver),
                        rd1_en=_has_src1(spec))
        shas[ver] = tmp.sha(ver)
    op = dvo.DveOp(name, spec, subdim=subdim, uops_sha=shas)
    dvo.OPS.append(op)
    dvo._SUB_OPCODE_FOR_NAME[name] = opcode
    dvo.CUSTOM_DVE_SPECS[name] = spec
    _my_ops[name] = op
    return op


def _ref_with_sum(body_fn):
    def _r(in0, in1, s0, s1, imm2):
        b = body_fn(in0, in1, s0, s1, imm2).astype(np.float32)
        return b, b.reshape(b.shape[0], -1).sum(-1, keepdims=True)
    return _r


def _registry():
    ops = {}
    ops["RELUPN"] = _register("ANT_RELUPN", Spec(
        body=relu(Src0) + relu(Zero - Src1),
        reference=lambda in0, in1, s0, s1, imm2:
            np.maximum(in0.astype(np.float32), 0)
            + np.maximum(-in1.astype(np.float32), 0),
    ))
    ops["COMB_ALPHA"] = _register("ANT_COMB_ALPHA", Spec(
        body=Src0 * C0 - Src1,
        reference=lambda in0, in1, s0, s1, imm2:
            in0.astype(np.float32) * s0 - in1.astype(np.float32),
    ))
    ops["RELU_MUL"] = _register("ANT_RELU_MUL", Spec(
        body=relu(Src0) * relu(Src1),
        reference=lambda in0, in1, s0, s1, imm2:
            np.maximum(in0.astype(np.float32), 0) * np.maximum(in1.astype(np.float32), 0),
    ))
    ops["SQ_ADD"] = _register("ANT_SQ_ADD", Spec(
        body=sq(Src0 + Src1),
        reference=lambda in0, in1, s0, s1, imm2:
            np.square(in0.astype(np.float32) + in1.astype(np.float32)),
    ))
    ops["SQ_ADD_S"] = _register("ANT_SQ_ADD_S", Spec(
        body=sq((Src0 + Src1) * C2),
        reference=lambda in0, in1, s0, s1, imm2:
            np.square((in0.astype(np.float32) + in1.astype(np.float32)) * imm2),
    ))
    ops["ARGSEL"] = _register("ANT_ARGSEL", Spec(
        body=select(sq(Src0) <= One, Src0, Src1),
        reference=lambda in0, in1, s0, s1, imm2:
            np.where(in0.astype(np.float32) ** 2 <= 1.0, in0, in1).astype(np.float32),
    ))
    _z = sq(Src0)
    ops["ATAN_P1"] = _register("ANT_ATAN_P1", Spec(
        body=(C0 * _z + C1) * _z + C2,
        reference=lambda in0, in1, s0, s1, imm2:
            ((s0 * in0.astype(np.float32) ** 2 + s1) * in0.astype(np.float32) ** 2 + imm2),
    ))
    _z2 = sq(Src0)
    ops["ATAN_P2"] = _register("ANT_ATAN_P2", Spec(
        body=(((Src1 * _z2 + C0) * _z2 + C1) * _z2 + C2) * Src0,
        reference=lambda in0, in1, s0, s1, imm2: (
            (((in1.astype(np.float32) * in0.astype(np.float32) ** 2 + s0)
              * in0.astype(np.float32) ** 2 + s1)
             * in0.astype(np.float32) ** 2 + imm2) * in0.astype(np.float32)),
    ))
    ops["RECON"] = _register("ANT_ATAN_RECON", Spec(
        body=select(sq(Src0) <= One, Src1,
                    select(Src0 >= Zero, C0, C1) - Src1),
        reference=lambda in0, in1, s0, s1, imm2: np.where(
            in0.astype(np.float32) ** 2 <= 1.0, in1,
            np.where(in0 >= 0, s0, s1) - in1).astype(np.float32),
    ))
    ops["LOSS_ACC"] = _register("ANT_LOSS_ACC", Spec(
        body=minn(relu(One - Src0), One) * Src1,
        accum=_op_add,
        reference=_ref_with_sum(
            lambda in0, in1, s0, s1, imm2:
                np.minimum(np.maximum(1.0 - in0.astype(np.float32), 0.0), 1.0)
                * in1.astype(np.float32)),
    ))
    return ops


# ------------------------------ program ------------------------------------
_cache = {}


def _build_program():
    if "nc" in _cache:
        return _cache["nc"]
    ops = _registry()
    RF = dvo.RECIPROCAL_APPROX_FAST
    RFC = dvo.RECIP_APPROX_FAST_CONSTS

    nc = bacc.Bacc("TRN2", debug=False, target_bir_lowering=False)

    def register_const_ap(dtype, value):
        tensor = nc.alloc_sbuf_tensor(f"const-{dtype.name}-{value}", [128, 1], dtype)
        nc.gpsimd.memset(tensor.ap(), value)
        nc.const_aps.aps[(dtype, value)] = tensor.ap()

    register_const_ap(F32, 1.0000001)
    nc.all_engine_barrier()
    dram = {}
    for nm in ("p0", "p1", "p2", "p3", "t0", "t1", "t2", "t3", "bn"):
        dram[nm] = nc.dram_tensor(nm, [P, F], F32, kind="ExternalInput").ap()
    dram["mk"] = nc.dram_tensor("mk", [P, F], U8, kind="ExternalInput").ap()
    out_acc = nc.dram_tensor("acc", [P, NCH], F32, kind="ExternalOutput").ap()

    DS = DT_SMALL

    # (name, dtype, engine, emit(env, dst)) — emitted in order; buffers are
    # assigned by last-use liveness below. engine: V=vector, A=act, G=gpsimd.
    def pipeline(nc, env, alloc, free_after):
        V, S, G = nc.vector, nc.scalar, nc.gpsimd
        Relu = mybir.ActivationFunctionType.Relu
        Squ = mybir.ActivationFunctionType.Square

        steps = []

        def step(name, dtype, fn, ins):
            steps.append((name, dtype, fn, ins))

        TT = mybir.AluOpType

        def vsub(a, b):
            return lambda d, e: V.tensor_sub(out=d[:], in0=e[a][:], in1=e[b][:])

        def vadd(a, b):
            return lambda d, e: V.tensor_add(out=d[:], in0=e[a][:], in1=e[b][:])

        def vmul(a, b):
            return lambda d, e: V.tensor_mul(out=d[:], in0=e[a][:], in1=e[b][:])

        def gsub(a, b):  # subtract on GPSIMD (frees DVE cycles)
            return lambda d, e: G.tensor_sub(out=d[:], in0=e[a][:], in1=e[b][:])

        def grelu(a):  # relu(x) on DVE tensor_scalar
            return lambda d, e: V.tensor_scalar(
                out=d[:], in0=e[a][:], scalar1=0.0, scalar2=None, op0=TT.max)

        def grelun(a):  # relu(-x) on DVE
            return lambda d, e: V.tensor_scalar(
                out=d[:], in0=e[a][:], scalar1=-1.0, scalar2=0.0,
                op0=TT.mult, op1=TT.max)

        def arelu(a, scale=1.0):  # relu(scale*x) on ACT
            return lambda d, e: S.activation(d[:], e[a][:], Relu, scale=scale)

        def cust(op, a, b=None, **kw):
            def _f(d, e):
                nc.vector._custom_dve(
                    op, out=d[:], in0=e[a][:],
                    in1=(e[b][:] if b is not None else None), **kw)
            return _f

        def recipf(a):
            return cust(RF, a, None, s0=RFC["s0"], s1=RFC["s1"], imm2=RFC["imm2"])

        # ---- prologue: fp32 in, DS out -------------------------------------
        step("d0", DS, vsub("p0", "t0"), ["p0", "t0"])
        step("d1", DS, gsub("p1", "t1"), ["p1", "t1"])
        step("d2", DS, vsub("p2", "t2"), ["p2", "t2"])
        step("d3", DS, gsub("p3", "t3"), ["p3", "t3"])
        step("wb", DS, gsub("t2", "t0"), ["t2", "t0"])
        step("hb", DS, gsub("t3", "t1"), ["t3", "t1"])
        step("wa", DS, vsub("p2", "p0"), ["p2", "p0"])
        step("ha", DS, vsub("p3", "p1"), ["p3", "p1"])
        # ---- relus ---------------------------------------------------------
        step("r0p", DS, grelu("d0"), ["d0"])
        step("r0n", DS, grelun("d0"), ["d0"])
        step("r2p", DS, grelu("d2"), ["d2"])
        step("r2n", DS, grelun("d2"), ["d2"])
        step("r1p", DS, arelu("d1"), ["d1"])
        step("r1n", DS, arelu("d1", -1.0), ["d1"])
        step("r3p", DS, arelu("d3"), ["d3"])
        step("r3n", DS, arelu("d3", -1.0), ["d3"])
        # ---- intersection --------------------------------------------------
        step("g1", DS, vadd("r0p", "r2n"), ["r0p", "r2n"])
        step("g2", DS, vadd("r1p", "r3n"), ["r1p", "r3n"])
        step("z1", DS, vsub("wb", "g1"), ["wb", "g1"])
        step("z2", DS, vsub("hb", "g2"), ["hb", "g2"])
        step("inter", DS, cust(ops["RELU_MUL"], "z1", "z2"), ["z1", "z2"])
        # ---- enclosing box / center distance (fp32: values overflow f16) ---
        step("h1", DS, vadd("r2p", "r0n"), ["r2p", "r0n"])
        step("h2", DS, vadd("r3p", "r1n"), ["r3p", "r1n"])
        step("cwv", DS, vadd("wb", "h1"), ["wb", "h1"])
        step("chv", DS, vadd("hb", "h2"), ["hb", "h2"])
        step("cw2", DS, lambda d, e: S.activation(
            d[:], e["cwv"][:], Squ, scale=0.0625), ["cwv"])
        step("ch2", DS, lambda d, e: S.activation(
            d[:], e["chv"][:], Squ, scale=0.0625), ["chv"])
        step("diag4", DS, vadd("cw2", "ch2"), ["cw2", "ch2"])
        step("rdiag", DS, recipf("diag4"), ["diag4"])
        step("cxv", DS, vadd("d0", "d2"), ["d0", "d2"])
        step("cyv", DS, vadd("d1", "d3"), ["d1", "d3"])
        step("cx2", DS, lambda d, e: S.activation(
            d[:], e["cxv"][:], Squ, scale=0.03125), ["cxv"])
        step("cy2", DS, lambda d, e: S.activation(
            d[:], e["cyv"][:], Squ, scale=0.03125), ["cyv"])
        step("cent4", DS, vadd("cx2", "cy2"), ["cx2", "cy2"])
        step("cd", DS, vmul("cent4", "rdiag"), ["cent4", "rdiag"])
        # ---- iou -----------------------------------------------------------
        step("A1", DS, vmul("wa", "ha"), ["wa", "ha"])
        step("A2", DS, vmul("wb", "hb"), ["wb", "hb"])
        step("u12", DS, vadd("A1", "A2"), ["A1", "A2"])
        step("union", DS, vsub("u12", "inter"), ["u12", "inter"])
        step("runion", DS, recipf("union"), ["union"])
        step("iou", DS, vmul("inter", "runion"), ["inter", "runion"])
        step("diou", DS, vsub("iou", "cd"), ["iou", "cd"])
        # ---- aspect-ratio term ---------------------------------------------
        step("n1", DS, vmul("wa", "hb"), ["wa", "hb"])
        step("n2", DS, vmul("wb", "ha"), ["wb", "ha"])
        step("num", DS, vsub("n1", "n2"), ["n1", "n2"])
        step("de1", DS, vmul("ha", "hb"), ["ha", "hb"])
        step("de2", DS, vmul("wa", "wb"), ["wa", "wb"])
        step("den", DS, vadd("de1", "de2"), ["de1", "de2"])
        step("n1", DS, vmul("wa", "hb"), ["wa", "hb"])
        step("n2", DS, vmul("wb", "ha"), ["wb", "ha"])
        step("num", DS, vsub("n1", "n2"), ["n1", "n2"])
        step("de1", DS, vmul("ha", "hb"), ["ha", "hb"])
        step("de2", DS, vmul("wa", "wb"), ["wa", "wb"])
        step("den", DS, vadd("de1", "de2"), ["de1", "de2"])
        step("rden", DS, recipf("den"), ["den"])
        step("T", DS, vmul("num", "rden"), ["num", "rden"])
        step("rT", DS, recipf("T"), ["T"])
        step("arg", DS, cust(ops["ARGSEL"], "T", "rT"), ["T", "rT"])
        step("pp1", DS, cust(ops["ATAN_P1"], "arg", None,
                             s0=ATAN_C[5], s1=ATAN_C[4], imm2=ATAN_C[3]), ["arg"])
        step("pp", DS, cust(ops["ATAN_P2"], "arg", "pp1",
                            s0=ATAN_C[2], s1=ATAN_C[1], imm2=ATAN_C[0]),
             ["arg", "pp1"])
        # p is (2/pi)-scaled, so the |T|>1 branch constant is sign(T)*1
        step("dth", DS, cust(ops["RECON"], "T", "pp",
                             s0=1.0, s1=-1.0), ["T", "pp"])
        step("v", DS, lambda d, e: S.activation(
            d[:], e["dth"][:], Squ, scale=2.0 / math.pi), ["dth"])
        step("v2", F32, lambda d, e: S.activation(d[:], e["v"][:], Squ), ["v"])
        # ---- alpha*v -------------------------------------------------------
        step("om", DS, lambda d, e: V.tensor_scalar(
            out=d[:], in0=e["iou"][:], scalar1=-1.0, scalar2=1.0000001,
            op0=TT.mult, op1=TT.add), ["iou"])
        step("vmi1", DS, vadd("v", "om"), ["v", "om"])
        step("rvd", F32, recipf("vmi1"), ["vmi1"])
        step("av", DS, vmul("v2", "rvd"), ["v2", "rvd"])
        step("ciou", DS, vsub("diou", "av"), ["diou", "av"])
        # ---- weighted clipped loss + reduce --------------------------------
        step("w", DS, vmul("mk", "bn"), ["mk", "bn"])
        return steps

    with tile.TileContext(nc) as tc:
        with tc.tile_pool(name="io", bufs=2) as pio, \
             tc.tile_pool(name="tmp", bufs=2) as ptmp, \
             tc.tile_pool(name="accp", bufs=1) as pacc:
            acc_sb = pacc.tile([P, NCH], F32, tag="acc_sb", name="acc_sb")
            for k in range(NCH):
                sl = slice(k * R, (k + 1) * R)
                env = {}
                for nm in ("p0", "p1", "p2", "p3", "t0", "t1", "t2", "t3"):
                    t = pio.tile([P, R], F32, tag=f"in_{nm}", name=f"in_{nm}")
                    nc.sync.dma_start(out=t[:], in_=dram[nm][:, sl])
                    env[nm] = t
                tb = pio.tile([P, R], DT_SMALL, tag="in_bn", name="in_bn")
                nc.gpsimd.dma_start(out=tb[:], in_=dram["bn"][:, sl])
                env["bn"] = tb
                tm = pio.tile([P, R], DT_SMALL, tag="in_mk", name="in_mk")
                nc.gpsimd.dma_start(out=tm[:], in_=dram["mk"][:, sl])
                env["mk"] = tm

                steps = pipeline(nc, env, None, None)
                # liveness: last step index using each name
                last_use = {}
                for i, (_, _, _, ins) in enumerate(steps):
                    for nm in ins:
                        last_use[nm] = i
                # buffer free-list per dtype
                free = {}
                owner = {}

                def take(dtype):
                    lst = free.setdefault(dtype, [])
                    if lst:
                        return lst.pop()
                    idx = take.counter = getattr(take, "counter", 0) + 1
                    return ptmp.tile([P, R], dtype, tag=f"tb_{dtype}_{idx}",
                                     name=f"tb_{dtype}_{idx}")

                for i, (nm, dtype, fn, ins) in enumerate(steps):
                    dst = take(dtype)
                    owner[nm] = (dst, dtype)
                    fn(dst, env)
                    env[nm] = dst
                    for used in ins:
                        if last_use.get(used) == i and used in owner:
                            bt, bd = owner.pop(used)
                            free.setdefault(bd, []).append(bt)

                # final fused loss+mask+reduce
                dummy = ptmp.tile([P, R], DT_SMALL, tag="dummy", name="dummy")
                nc.vector._custom_dve(
                    _my_ops["ANT_LOSS_ACC"], out=dummy[:],
                    in0=env["ciou"][:], in1=env["w"][:],
                    accum_out=acc_sb[:, k:k + 1])
            nc.sync.dma_start(out=out_acc[:], in_=acc_sb[:])

    nc.compile()
    _cache["nc"] = nc
    return nc


# ------------------------------- host side ---------------------------------

def _shard_inputs(predicts_bbox, targets_bbox, valid_masks, box_norm):
    in_maps = []
    pr = np.asarray(predicts_bbox, dtype=np.float32).reshape(B, A, 4)
    tg = np.asarray(targets_bbox, dtype=np.float32).reshape(B, A, 4)
    vm = np.asarray(valid_masks)
    bn = np.asarray(box_norm, dtype=np.float32)
    for c in range(N_CORES):
        rows = slice(c * B_LOC, (c + 1) * B_LOC)
        pc = pr[rows].reshape(E, 4)
        tc_ = tg[rows].reshape(E, 4)
        m = {}
        for i in range(4):
            m[f"p{i}"] = np.ascontiguousarray(pc[:, i]).reshape(P, F)
            m[f"t{i}"] = np.ascontiguousarray(tc_[:, i]).reshape(P, F)
        m["bn"] = np.ascontiguousarray(bn[rows]).reshape(P, F)
        m["mk"] = np.ascontiguousarray(
            vm[rows]).reshape(P, F).astype(np.uint8)
        in_maps.append(m)
    return in_maps


def kernel(predicts_bbox, targets_bbox, valid_masks, box_norm, cls_norm):
    nc = _build_program()
    in_maps = _shard_inputs(predicts_bbox, targets_bbox, valid_masks, box_norm)
    res = bass_utils.run_bass_kernel_spmd(nc, in_maps, core_ids=list(range(N_CORES)))
    total = np.float64(0.0)
    for c in range(N_CORES):
        total += res.results[c]["acc"].astype(np.float64).sum()
    out = np.float32(total / np.float64(np.asarray(cls_norm)))
    return np.asarray(out, dtype=np.float32)


# revision 28
# speedup vs baseline: 1.0197x; 1.0197x over previous
"""Trainium2 Bass kernel for nn_BoxLoss (masked weighted CIoU loss).

Contract: kernel(**inputs) takes the FULL unsharded inputs
  predicts_bbox [128, 33600, 4] f32, targets_bbox [128, 33600, 4] f32,
  valid_masks [128, 33600] bool, box_norm [128, 33600] f32, cls_norm () f32
and returns the FULL scalar output, sharding batch rows across 8 NeuronCores
internally (pure data parallel, per the sharding hint).

Per-core layout: 16 batch rows x 33600 anchors = 537600 elements laid out
[128 partitions, 4200] (partition-major, each partition owns a contiguous
span). Box coords are de-interleaved on host into planar channels so every
device-side access is contiguous.

Math notes (exact reformulation of the reference):
  d_c  = p_c - t_c ;  wb = t2-t0, hb = t3-t1, wa = p2-p0, ha = p3-p1
  iw   = wb - relu(-d2) - relu(d0)       (== min(p2,t2) - max(p0,t0))
  cw   = wb + relu(d2) + relu(-d0)       (== max(p2,t2) - min(p0,t0))
  cent*4 = (d0+d2)^2 + (d1+d3)^2 ;  diag*4 = (2cw)^2 + (2ch)^2
  => cent*0.25/diag = cent4 / diag4
  atan(u)-atan(v) = atan(T), T=(wa*hb - wb*ha)/(ha*hb + wa*wb), via
  |T|<=1 ? atan(T) : sign(T)*pi/2 - atan(1/T), atan by deg-11 minimax poly.
  Non-overlapping pairs give inter=0 -> ciou = -cd-av < 0 -> loss contrib
  is exactly w (the clip), so fp16 intermediates only perturb overlapping
  pairs (small relative coords) when DT_SMALL = float16.
"""

import sys

if "/opt/trn_rl_repo" not in sys.path:
    sys.path.insert(0, "/opt/trn_rl_repo")

import math
import numpy as np

import concourse.bacc as bacc
from concourse import mybir, tile
from concourse import bass_utils
from concourse import dve_ops as dvo
from concourse.dve_spec import (
    Spec, Src0, Src1, C0, C1, C2, Zero, One, AluOp,
    relu, sq, maxx, minn, select, lower, _has_src1,
)
from concourse.dve_uop import DveOpSpec
from operator import add as _op_add

# ------------------------------- config ------------------------------------
B, A = 128, 33600
N_CORES = 8
B_LOC = B // N_CORES                # 16 batch rows per core
E = B_LOC * A                       # 537600 elements per core
P = 128                             # partitions
F = E // P                          # 4200 free elements per partition
R = 1050                            # chunk free size (divides F)
NCH = F // R

F32 = mybir.dt.float32
F16 = mybir.dt.float16
U8 = mybir.dt.uint8

# dtype of the "small" intermediate chain. float32 is the safe default;
# float16 doubles stock DVE tensor_tensor throughput.
DT_SMALL = F16

HALF_PI = math.pi / 2.0
# minimax atan(x) ~ x*(c0 + c1 z + ... + c5 z^5), z=x^2, |x|<=1, err 1.7e-6
ATAN_C = [0.9999772562021794, -0.3326237246324494, 0.19354622050707823,
          -0.11644164122245204, 0.05266424416536723, -0.011725888127135233]

# --------------------------- custom DVE ops --------------------------------
_my_ops = {}


def _register(name, spec, subdim=False):
    if name in _my_ops:
        return _my_ops[name]
    existing = {op.name: op for op in dvo.OPS}
    if name in existing:
        _my_ops[name] = existing[name]
        return existing[name]
    opcode = dvo._CUSTOM_DVE_ROW_BASE + len(dvo.OPS)
    shas = {}
    for ver in ("v3", "v4"):
        tmp = DveOpSpec(name=name, opcode=opcode, uops=lower(spec, ver=ver),
                        rd1_en=_has_src1(spec))
        shas[ver] = tmp.sha(ver)
    op = dvo.DveOp(name, spec, subdim=subdim, uops_sha=shas)
    dvo.OPS.append(op)
    dvo._SUB_OPCODE_FOR_NAME[name] = opcode
    dvo.CUSTOM_DVE_SPECS[name] = spec
    _my_ops[name] = op
    return op


def _ref_with_sum(body_fn):
    def _r(in0, in1, s0, s1, imm2):
        b = body_fn(in0, in1, s0, s1, imm2).astype(np.float32)
        return b, b.reshape(b.shape[0], -1).sum(-1, keepdims=True)
    return _r


def _registry():
    ops = {}
    ops["RELUPN"] = _register("ANT_RELUPN", Spec(
        body=relu(Src0) + relu(Zero - Src1),
        reference=lambda in0, in1, s0, s1, imm2:
            np.maximum(in0.astype(np.float32), 0)
            + np.maximum(-in1.astype(np.float32), 0),
    ))
    ops["COMB_ALPHA"] = _register("ANT_COMB_ALPHA", Spec(
        body=Src0 * C0 - Src1,
        reference=lambda in0, in1, s0, s1, imm2:
            in0.astype(np.float32) * s0 - in1.astype(np.float32),
    ))
    ops["RELU_MUL"] = _register("ANT_RELU_MUL", Spec(
        body=relu(Src0) * relu(Src1),
        reference=lambda in0, in1, s0, s1, imm2:
            np.maximum(in0.astype(np.float32), 0) * np.maximum(in1.astype(np.float32), 0),
    ))
    ops["SQ_ADD"] = _register("ANT_SQ_ADD", Spec(
        body=sq(Src0 + Src1),
        reference=lambda in0, in1, s0, s1, imm2:
            np.square(in0.astype(np.float32) + in1.astype(np.float32)),
    ))
    ops["SQ_ADD_S"] = _register("ANT_SQ_ADD_S", Spec(
        body=sq((Src0 + Src1) * C2),
        reference=lambda in0, in1, s0, s1, imm2:
            np.square((in0.astype(np.float32) + in1.astype(np.float32)) * imm2),
    ))
    ops["ARGSEL"] = _register("ANT_ARGSEL", Spec(
        body=select(sq(Src0) <= One, Src0, Src1),
        reference=lambda in0, in1, s0, s1, imm2:
            np.where(in0.astype(np.float32) ** 2 <= 1.0, in0, in1).astype(np.float32),
    ))
    _z = sq(Src0)
    ops["ATAN_P1"] = _register("ANT_ATAN_P1", Spec(
        body=(C0 * _z + C1) * _z + C2,
        reference=lambda in0, in1, s0, s1, imm2:
            ((s0 * in0.astype(np.float32) ** 2 + s1) * in0.astype(np.float32) ** 2 + imm2),
    ))
    _z2 = sq(Src0)
    ops["ATAN_P2"] = _register("ANT_ATAN_P2", Spec(
        body=(((Src1 * _z2 + C0) * _z2 + C1) * _z2 + C2) * Src0,
        reference=lambda in0, in1, s0, s1, imm2: (
            (((in1.astype(np.float32) * in0.astype(np.float32) ** 2 + s0)
              * in0.astype(np.float32) ** 2 + s1)
             * in0.astype(np.float32) ** 2 + imm2) * in0.astype(np.float32)),
    ))
    ops["RECON"] = _register("ANT_ATAN_RECON", Spec(
        body=select(sq(Src0) <= One, Src1,
                    select(Src0 >= Zero, C0, C1) - Src1),
        reference=lambda in0, in1, s0, s1, imm2: np.where(
            in0.astype(np.float32) ** 2 <= 1.0, in1,
            np.where(in0 >= 0, s0, s1) - in1).astype(np.float32),
    ))
    ops["LOSS_ACC"] = _register("ANT_LOSS_ACC", Spec(
        body=minn(relu(One - Src0), One) * Src1,
        accum=_op_add,
        reference=_ref_with_sum(
            lambda in0, in1, s0, s1, imm2:
                np.minimum(np.maximum(1.0 - in0.astype(np.float32), 0.0), 1.0)
                * in1.astype(np.float32)),
    ))
    return ops


# ------------------------------ program ------------------------------------
_cache = {}


def _build_program():
    if "nc" in _cache:
        return _cache["nc"]
    ops = _registry()
    RF = dvo.RECIPROCAL_APPROX_FAST
    RFC = dvo.RECIP_APPROX_FAST_CONSTS

    nc = bacc.Bacc("TRN2", debug=False, target_bir_lowering=False)

    def register_const_ap(dtype, value):
        tensor = nc.alloc_sbuf_tensor(f"const-{dtype.name}-{value}", [128, 1], dtype)
        nc.gpsimd.memset(tensor.ap(), value)
        nc.const_aps.aps[(dtype, value)] = tensor.ap()

    register_const_ap(F32, 1.0000001)
    nc.all_engine_barrier()
    dram = {}
    for nm in ("p0", "p1", "p2", "p3", "t0", "t1", "t2", "t3", "bn"):
        dram[nm] = nc.dram_tensor(nm, [P, F], F32, kind="ExternalInput").ap()
    dram["mk"] = nc.dram_tensor("mk", [P, F], U8, kind="ExternalInput").ap()
    out_acc = nc.dram_tensor("acc", [P, NCH], F32, kind="ExternalOutput").ap()

    DS = DT_SMALL

    # (name, dtype, engine, emit(env, dst)) — emitted in order; buffers are
    # assigned by last-use liveness below. engine: V=vector, A=act, G=gpsimd.
    def pipeline(nc, env, alloc, free_after):
        V, S, G = nc.vector, nc.scalar, nc.gpsimd
        Relu = mybir.ActivationFunctionType.Relu
        Squ = mybir.ActivationFunctionType.Square

        steps = []

        def step(name, dtype, fn, ins):
            steps.append((name, dtype, fn, ins))

        TT = mybir.AluOpType

        def vsub(a, b):
            return lambda d, e: V.tensor_sub(out=d[:], in0=e[a][:], in1=e[b][:])

        def vadd(a, b):
            return lambda d, e: V.tensor_add(out=d[:], in0=e[a][:], in1=e[b][:])

        def vmul(a, b):
            return lambda d, e: V.tensor_mul(out=d[:], in0=e[a][:], in1=e[b][:])

        def gsub(a, b):  # subtract on GPSIMD (frees DVE cycles)
            return lambda d, e: G.tensor_sub(out=d[:], in0=e[a][:], in1=e[b][:])

        def grelu(a):  # relu(x) on DVE tensor_scalar
            return lambda d, e: V.tensor_scalar(
                out=d[:], in0=e[a][:], scalar1=0.0, scalar2=None, op0=TT.max)

        def grelun(a):  # relu(-x) on DVE
            return lambda d, e: V.tensor_scalar(
                out=d[:], in0=e[a][:], scalar1=-1.0, scalar2=0.0,
                op0=TT.mult, op1=TT.max)

        def arelu(a, scale=1.0):  # relu(scale*x) on ACT
            return lambda d, e: S.activation(d[:], e[a][:], Relu, scale=scale)

        def cust(op, a, b=None, **kw):
            def _f(d, e):
                nc.vector._custom_dve(
                    op, out=d[:], in0=e[a][:],
                    in1=(e[b][:] if b is not None else None), **kw)
            return _f

        def recipf(a):
            return cust(RF, a, None, s0=RFC["s0"], s1=RFC["s1"], imm2=RFC["imm2"])

        # ---- prologue: fp32 in, DS out -------------------------------------
        step("d0", DS, vsub("p0", "t0"), ["p0", "t0"])
        step("d1", DS, gsub("p1", "t1"), ["p1", "t1"])
        step("d2", DS, vsub("p2", "t2"), ["p2", "t2"])
        step("d3", DS, gsub("p3", "t3"), ["p3", "t3"])
        step("wb", DS, gsub("t2", "t0"), ["t2", "t0"])
        step("hb", DS, gsub("t3", "t1"), ["t3", "t1"])
        step("wa", DS, vsub("p2", "p0"), ["p2", "p0"])
        step("ha", DS, vsub("p3", "p1"), ["p3", "p1"])
        # ---- relus ---------------------------------------------------------
        step("r0p", DS, grelu("d0"), ["d0"])
        step("r0n", DS, grelun("d0"), ["d0"])
        step("r2p", DS, grelu("d2"), ["d2"])
        step("r2n", DS, grelun("d2"), ["d2"])
        step("r1p", DS, arelu("d1"), ["d1"])
        step("r1n", DS, arelu("d1", -1.0), ["d1"])
        step("r3p", DS, arelu("d3"), ["d3"])
        step("r3n", DS, arelu("d3", -1.0), ["d3"])
        # ---- intersection --------------------------------------------------
        step("g1", DS, vadd("r0p", "r2n"), ["r0p", "r2n"])
        step("g2", DS, vadd("r1p", "r3n"), ["r1p", "r3n"])
        step("z1", DS, vsub("wb", "g1"), ["wb", "g1"])
        step("z2", DS, vsub("hb", "g2"), ["hb", "g2"])
        step("inter", DS, cust(ops["RELU_MUL"], "z1", "z2"), ["z1", "z2"])
        # ---- enclosing box / center distance (fp32: values overflow f16) ---
        step("h1", DS, vadd("r2p", "r0n"), ["r2p", "r0n"])
        step("h2", DS, vadd("r3p", "r1n"), ["r3p", "r1n"])
        step("cwv", DS, vadd("wb", "h1"), ["wb", "h1"])
        step("chv", DS, vadd("hb", "h2"), ["hb", "h2"])
        step("cw2", DS, lambda d, e: S.activation(
            d[:], e["cwv"][:], Squ, scale=0.0625), ["cwv"])
        step("ch2", DS, lambda d, e: S.activation(
            d[:], e["chv"][:], Squ, scale=0.0625), ["chv"])
        step("diag4", DS, vadd("cw2", "ch2"), ["cw2", "ch2"])
        step("rdiag", DS, recipf("diag4"), ["diag4"])
        step("cxv", DS, vadd("d0", "d2"), ["d0", "d2"])
        step("cyv", DS, vadd("d1", "d3"), ["d1", "d3"])
        step("cx2", DS, lambda d, e: S.activation(
            d[:], e["cxv"][:], Squ, scale=0.03125), ["cxv"])
        step("cy2", DS, lambda d, e: S.activation(
            d[:], e["cyv"][:], Squ, scale=0.03125), ["cyv"])
        step("cent4", DS, vadd("cx2", "cy2"), ["cx2", "cy2"])
        step("cd", DS, vmul("cent4", "rdiag"), ["cent4", "rdiag"])
        # ---- iou -----------------------------------------------------------
        step("A1", DS, vmul("wa", "ha"), ["wa", "ha"])
        step("A2", DS, vmul("wb", "hb"), ["wb", "hb"])
        step("u12", DS, vadd("A1", "A2"), ["A1", "A2"])
        step("union", DS, vsub("u12", "inter"), ["u12", "inter"])
        step("runion", DS, recipf("union"), ["union"])
        step("iou", DS, vmul("inter", "runion"), ["inter", "runion"])
        step("diou", DS, vsub("iou", "cd"), ["iou", "cd"])
        # ---- aspect-ratio term ---------------------------------------------
        step("n1", DS, vmul("wa", "hb"), ["wa", "hb"])
        step("n2", DS, vmul("wb", "ha"), ["wb", "ha"])
        step("num", DS, vsub("n1", "n2"), ["n1", "n2"])
        step("de1", DS, vmul("ha", "hb"), ["ha", "hb"])
        step("de2", DS, vmul("wa", "wb"), ["wa", "wb"])
        step("den", DS, vadd("de1", "de2"), ["de1", "de2"])
        step("n1", DS, vmul("wa", "hb"), ["wa", "hb"])
        step("n2", DS, vmul("wb", "ha"), ["wb", "ha"])
        step("num", DS, vsub("n1", "n2"), ["n1", "n2"])
        step("de1", DS, vmul("ha", "hb"), ["ha", "hb"])
        step("de2", DS, vmul("wa", "wb"), ["wa", "wb"])
        step("den", DS, vadd("de1", "de2"), ["de1", "de2"])
        step("rden", DS, recipf("den"), ["den"])
        step("T", DS, vmul("num", "rden"), ["num", "rden"])
        step("rT", DS, recipf("T"), ["T"])
        step("arg", DS, cust(ops["ARGSEL"], "T", "rT"), ["T", "rT"])
        step("pp1", DS, cust(ops["ATAN_P1"], "arg", None,
                             s0=ATAN_C[5], s1=ATAN_C[4], imm2=ATAN_C[3]), ["arg"])
        step("pp", DS, cust(ops["ATAN_P2"], "arg", "pp1",
                            s0=ATAN_C[2], s1=ATAN_C[1], imm2=ATAN_C[0]),
             ["arg", "pp1"])
        # p is (2/pi)-scaled, so the |T|>1 branch constant is sign(T)*1
        step("dth", DS, cust(ops["RECON"], "T", "pp",
                             s0=1.0, s1=-1.0), ["T", "pp"])
        step("v", DS, lambda d, e: S.activation(
            d[:], e["dth"][:], Squ, scale=2.0 / math.pi), ["dth"])
        step("v2", F32, lambda d, e: S.activation(d[:], e["v"][:], Squ), ["v"])
        # ---- alpha*v -------------------------------------------------------
        step("om", DS, lambda d, e: V.tensor_scalar(
            out=d[:], in0=e["iou"][:], scalar1=-1.0, scalar2=1.0000001,
            op0=TT.mult, op1=TT.add), ["iou"])
        step("vmi1", DS, vadd("v", "om"), ["v", "om"])
        step("rvd", F32, recipf("vmi1"), ["vmi1"])
        step("av", DS, vmul("v2", "rvd"), ["v2", "rvd"])
        step("ciou", DS, vsub("diou", "av"), ["diou", "av"])
        # ---- weighted clipped loss + reduce --------------------------------
        step("w", DS, vmul("mk", "bn"), ["mk", "bn"])
        return steps

    with tile.TileContext(nc) as tc:
        with tc.tile_pool(name="io", bufs=2) as pio, \
             tc.tile_pool(name="tmp", bufs=2) as ptmp, \
             tc.tile_pool(name="accp", bufs=1) as pacc:
            acc_sb = pacc.tile([P, NCH], F32, tag="acc_sb", name="acc_sb")
            for k in range(NCH):
                sl = slice(k * R, (k + 1) * R)
                env = {}
                for nm in ("p0", "p1", "p2", "p3", "t0", "t1", "t2", "t3"):
                    t = pio.tile([P, R], F32, tag=f"in_{nm}", name=f"in_{nm}")
                    nc.sync.dma_start(out=t[:], in_=dram[nm][:, sl])
                    env[nm] = t
                tb = pio.tile([P, R], DT_SMALL, tag="in_bn", name="in_bn")
                nc.gpsimd.dma_start(out=tb[:], in_=dram["bn"][:, sl])
                env["bn"] = tb
                tm = pio.tile([P, R], DT_SMALL, tag="in_mk", name="in_mk")
                nc.gpsimd.dma_start(out=tm[:], in_=dram["mk"][:, sl])
                env["mk"] = tm

                steps = pipeline(nc, env, None, None)
                # liveness: last step index using each name
                last_use = {}
                for i, (_, _, _, ins) in enumerate(steps):
                    for nm in ins:
                        last_use[nm] = i
                # buffer free-list per dtype
                free = {}
                owner = {}

                def take(dtype):
                    lst = free.setdefault(dtype, [])
                    if lst:
                        return lst.pop()
                    idx = take.counter = getattr(take, "counter", 0) + 1
                    return ptmp.tile([P, R], dtype, tag=f"tb_{dtype}_{idx}",
                                     name=f"tb_{dtype}_{idx}")

                for i, (nm, dtype, fn, ins) in enumerate(steps):
                    dst = take(dtype)
                    owner[nm] = (dst, dtype)
                    fn(dst, env)
                    env[nm] = dst
                    for used in ins:
                        if last_use.get(used) == i and used in owner:
                            bt, bd = owner.pop(used)
                            free.setdefault(bd, []).append(bt)

                # final fused loss+mask+reduce
                dummy = ptmp.tile([P, R], DT_SMALL, tag="dummy", name="dummy")
                nc.vector._custom_dve(
                    _my_ops["ANT_LOSS_ACC"], out=dummy[:],
                    in0=env["ciou"][:], in1=env["w"][:],
                    accum_out=acc_sb[:, k:k + 1])
            nc.sync.dma_start(out=out_acc[:], in_=acc_sb[:])

    nc.compile()
    _cache["nc"] = nc
    return nc


# ------------------------------- host side ---------------------------------

def _shard_inputs(predicts_bbox, targets_bbox, valid_masks, box_norm):
    in_maps = []
    pr = np.asarray(predicts_bbox, dtype=np.float32).reshape(B, A, 4)
    tg = np.asarray(targets_bbox, dtype=np.float32).reshape(B, A, 4)
    vm = np.asarray(valid_masks)
    bn = np.asarray(box_norm, dtype=np.float32)
    for c in range(N_CORES):
        rows = slice(c * B_LOC, (c + 1) * B_LOC)
        pc = pr[rows].reshape(E, 4)
        tc_ = tg[rows].reshape(E, 4)
        m = {}
        for i in range(4):
            m[f"p{i}"] = np.ascontiguousarray(pc[:, i]).reshape(P, F)
            m[f"t{i}"] = np.ascontiguousarray(tc_[:, i]).reshape(P, F)
        m["bn"] = np.ascontiguousarray(bn[rows]).reshape(P, F)
        m["mk"] = np.ascontiguousarray(
            vm[rows]).reshape(P, F).astype(np.uint8)
        in_maps.append(m)
    return in_maps


def kernel(predicts_bbox, targets_bbox, valid_masks, box_norm, cls_norm):
    nc = _build_program()
    in_maps = _shard_inputs(predicts_bbox, targets_bbox, valid_masks, box_norm)
    res = bass_utils.run_bass_kernel_spmd(nc, in_maps, core_ids=list(range(N_CORES)))
    total = np.float64(0.0)
    for c in range(N_CORES):
        total += res.results[c]["acc"].astype(np.float64).sum()
    out = np.float32(total / np.float64(np.asarray(cls_norm)))
    return np.asarray(out, dtype=np.float32)


# revision 31
# speedup vs baseline: 1.0409x; 1.0208x over previous
"""Trainium2 Bass kernel for nn_BoxLoss (masked weighted CIoU loss).

Contract: kernel(**inputs) takes the FULL unsharded inputs
  predicts_bbox [128, 33600, 4] f32, targets_bbox [128, 33600, 4] f32,
  valid_masks [128, 33600] bool, box_norm [128, 33600] f32, cls_norm () f32
and returns the FULL scalar output, sharding batch rows across 8 NeuronCores
internally (pure data parallel, per the sharding hint).

Per-core layout: 16 batch rows x 33600 anchors = 537600 elements laid out
[128 partitions, 4200] (partition-major, each partition owns a contiguous
span). Box coords are de-interleaved on host into planar channels so every
device-side access is contiguous.

Math notes (exact reformulation of the reference):
  d_c  = p_c - t_c ;  wb = t2-t0, hb = t3-t1, wa = p2-p0, ha = p3-p1
  iw   = wb - relu(-d2) - relu(d0)       (== min(p2,t2) - max(p0,t0))
  cw   = wb + relu(d2) + relu(-d0)       (== max(p2,t2) - min(p0,t0))
  cent*4 = (d0+d2)^2 + (d1+d3)^2 ;  diag*4 = (2cw)^2 + (2ch)^2
  => cent*0.25/diag = cent4 / diag4
  atan(u)-atan(v) = atan(T), T=(wa*hb - wb*ha)/(ha*hb + wa*wb), via
  |T|<=1 ? atan(T) : sign(T)*pi/2 - atan(1/T), atan by deg-11 minimax poly.
  Non-overlapping pairs give inter=0 -> ciou = -cd-av < 0 -> loss contrib
  is exactly w (the clip), so fp16 intermediates only perturb overlapping
  pairs (small relative coords) when DT_SMALL = float16.
"""

import sys

if "/opt/trn_rl_repo" not in sys.path:
    sys.path.insert(0, "/opt/trn_rl_repo")

import math
import numpy as np

import concourse.bacc as bacc
from concourse import mybir, tile
from concourse import bass_utils
from concourse import dve_ops as dvo
from concourse.dve_spec import (
    Spec, Src0, Src1, C0, C1, C2, Zero, One, AluOp,
    relu, sq, maxx, minn, select, lower, _has_src1,
)
from concourse.dve_uop import DveOpSpec
from operator import add as _op_add

# ------------------------------- config ------------------------------------
B, A = 128, 33600
N_CORES = 8
B_LOC = B // N_CORES                # 16 batch rows per core
E = B_LOC * A                       # 537600 elements per core
P = 128                             # partitions
F = E // P                          # 4200 free elements per partition
R = 1050                            # chunk free size (divides F)
NCH = F // R

F32 = mybir.dt.float32
F16 = mybir.dt.float16
U8 = mybir.dt.uint8

# dtype of the "small" intermediate chain. float32 is the safe default;
# float16 doubles stock DVE tensor_tensor throughput.
DT_SMALL = F16

HALF_PI = math.pi / 2.0
# minimax atan(x) ~ x*(c0 + c1 z + ... + c5 z^5), z=x^2, |x|<=1, err 1.7e-6
ATAN_C = [0.9999772562021794, -0.3326237246324494, 0.19354622050707823,
          -0.11644164122245204, 0.05266424416536723, -0.011725888127135233]

# --------------------------- custom DVE ops --------------------------------
_my_ops = {}


def _register(name, spec, subdim=False):
    if name in _my_ops:
        return _my_ops[name]
    existing = {op.name: op for op in dvo.OPS}
    if name in existing:
        _my_ops[name] = existing[name]
        return existing[name]
    opcode = dvo._CUSTOM_DVE_ROW_BASE + len(dvo.OPS)
    shas = {}
    for ver in ("v3", "v4"):
        tmp = DveOpSpec(name=name, opcode=opcode, uops=lower(spec, ver=ver),
                        rd1_en=_has_src1(spec))
        shas[ver] = tmp.sha(ver)
    op = dvo.DveOp(name, spec, subdim=subdim, uops_sha=shas)
    dvo.OPS.append(op)
    dvo._SUB_OPCODE_FOR_NAME[name] = opcode
    dvo.CUSTOM_DVE_SPECS[name] = spec
    _my_ops[name] = op
    return op


def _ref_with_sum(body_fn):
    def _r(in0, in1, s0, s1, imm2):
        b = body_fn(in0, in1, s0, s1, imm2).astype(np.float32)
        return b, b.reshape(b.shape[0], -1).sum(-1, keepdims=True)
    return _r


def _registry():
    ops = {}
    ops["RELUPN"] = _register("ANT_RELUPN", Spec(
        body=relu(Src0) + relu(Zero - Src1),
        reference=lambda in0, in1, s0, s1, imm2:
            np.maximum(in0.astype(np.float32), 0)
            + np.maximum(-in1.astype(np.float32), 0),
    ))
    ops["COMB_ALPHA"] = _register("ANT_COMB_ALPHA", Spec(
        body=Src0 * C0 - Src1,
        reference=lambda in0, in1, s0, s1, imm2:
            in0.astype(np.float32) * s0 - in1.astype(np.float32),
    ))
    ops["RELU_MUL"] = _register("ANT_RELU_MUL", Spec(
        body=relu(Src0) * relu(Src1),
        reference=lambda in0, in1, s0, s1, imm2:
            np.maximum(in0.astype(np.float32), 0) * np.maximum(in1.astype(np.float32), 0),
    ))
    ops["SQ_ADD"] = _register("ANT_SQ_ADD", Spec(
        body=sq(Src0 + Src1),
        reference=lambda in0, in1, s0, s1, imm2:
            np.square(in0.astype(np.float32) + in1.astype(np.float32)),
    ))
    ops["SQ_ADD_S"] = _register("ANT_SQ_ADD_S", Spec(
        body=sq((Src0 + Src1) * C2),
        reference=lambda in0, in1, s0, s1, imm2:
            np.square((in0.astype(np.float32) + in1.astype(np.float32)) * imm2),
    ))
    ops["ARGSEL"] = _register("ANT_ARGSEL", Spec(
        body=select(sq(Src0) <= One, Src0, Src1),
        reference=lambda in0, in1, s0, s1, imm2:
            np.where(in0.astype(np.float32) ** 2 <= 1.0, in0, in1).astype(np.float32),
    ))
    _z = sq(Src0)
    ops["ATAN_P1"] = _register("ANT_ATAN_P1", Spec(
        body=(C0 * _z + C1) * _z + C2,
        reference=lambda in0, in1, s0, s1, imm2:
            ((s0 * in0.astype(np.float32) ** 2 + s1) * in0.astype(np.float32) ** 2 + imm2),
    ))
    _z2 = sq(Src0)
    ops["ATAN_P2"] = _register("ANT_ATAN_P2", Spec(
        body=(((Src1 * _z2 + C0) * _z2 + C1) * _z2 + C2) * Src0,
        reference=lambda in0, in1, s0, s1, imm2: (
            (((in1.astype(np.float32) * in0.astype(np.float32) ** 2 + s0)
              * in0.astype(np.float32) ** 2 + s1)
             * in0.astype(np.float32) ** 2 + imm2) * in0.astype(np.float32)),
    ))
    ops["RECON"] = _register("ANT_ATAN_RECON", Spec(
        body=select(sq(Src0) <= One, Src1,
                    select(Src0 >= Zero, C0, C1) - Src1),
        reference=lambda in0, in1, s0, s1, imm2: np.where(
            in0.astype(np.float32) ** 2 <= 1.0, in1,
            np.where(in0 >= 0, s0, s1) - in1).astype(np.float32),
    ))
    ops["LOSS_ACC"] = _register("ANT_LOSS_ACC", Spec(
        body=minn(relu(One - Src0), One) * Src1,
        accum=_op_add,
        reference=_ref_with_sum(
            lambda in0, in1, s0, s1, imm2:
                np.minimum(np.maximum(1.0 - in0.astype(np.float32), 0.0), 1.0)
                * in1.astype(np.float32)),
    ))
    return ops


# ------------------------------ program ------------------------------------
_cache = {}


def _build_program():
    if "nc" in _cache:
        return _cache["nc"]
    ops = _registry()
    RF = dvo.RECIPROCAL_APPROX_FAST
    RFC = dvo.RECIP_APPROX_FAST_CONSTS

    nc = bacc.Bacc("TRN2", debug=False, target_bir_lowering=False)

    def register_const_ap(dtype, value):
        tensor = nc.alloc_sbuf_tensor(f"const-{dtype.name}-{value}", [128, 1], dtype)
        nc.gpsimd.memset(tensor.ap(), value)
        nc.const_aps.aps[(dtype, value)] = tensor.ap()

    register_const_ap(F32, 1.0000001)
    nc.all_engine_barrier()
    dram = {}
    for nm in ("p0", "p1", "p2", "p3", "t0", "t1", "t2", "t3", "bn"):
        dram[nm] = nc.dram_tensor(nm, [P, F], F32, kind="ExternalInput").ap()
    dram["mk"] = nc.dram_tensor("mk", [P, F], U8, kind="ExternalInput").ap()
    out_acc = nc.dram_tensor("acc", [P, NCH], F32, kind="ExternalOutput").ap()

    DS = DT_SMALL

    # (name, dtype, engine, emit(env, dst)) — emitted in order; buffers are
    # assigned by last-use liveness below. engine: V=vector, A=act, G=gpsimd.
    def pipeline(nc, env, alloc, free_after):
        V, S, G = nc.vector, nc.scalar, nc.gpsimd
        Relu = mybir.ActivationFunctionType.Relu
        Squ = mybir.ActivationFunctionType.Square

        steps = []

        def step(name, dtype, fn, ins):
            steps.append((name, dtype, fn, ins))

        TT = mybir.AluOpType

        def vsub(a, b):
            return lambda d, e: V.tensor_sub(out=d[:], in0=e[a][:], in1=e[b][:])

        def vadd(a, b):
            return lambda d, e: V.tensor_add(out=d[:], in0=e[a][:], in1=e[b][:])

        def vmul(a, b):
            return lambda d, e: V.tensor_mul(out=d[:], in0=e[a][:], in1=e[b][:])

        def gsub(a, b):  # subtract on GPSIMD (frees DVE cycles)
            return lambda d, e: G.tensor_sub(out=d[:], in0=e[a][:], in1=e[b][:])

        def grelu(a):  # relu(x) on DVE tensor_scalar
            return lambda d, e: V.tensor_scalar(
                out=d[:], in0=e[a][:], scalar1=0.0, scalar2=None, op0=TT.max)

        def grelun(a):  # relu(-x) on DVE
            return lambda d, e: V.tensor_scalar(
                out=d[:], in0=e[a][:], scalar1=-1.0, scalar2=0.0,
                op0=TT.mult, op1=TT.max)

        def arelu(a, scale=1.0):  # relu(scale*x) on ACT
            return lambda d, e: S.activation(d[:], e[a][:], Relu, scale=scale)

        def cust(op, a, b=None, **kw):
            def _f(d, e):
                nc.vector._custom_dve(
                    op, out=d[:], in0=e[a][:],
                    in1=(e[b][:] if b is not None else None), **kw)
            return _f

        def recipf(a):
            return cust(RF, a, None, s0=RFC["s0"], s1=RFC["s1"], imm2=RFC["imm2"])

        # ---- prologue: fp32 in, DS out -------------------------------------
        step("d0", DS, vsub("p0", "t0"), ["p0", "t0"])
        step("d1", DS, gsub("p1", "t1"), ["p1", "t1"])
        step("d2", DS, vsub("p2", "t2"), ["p2", "t2"])
        step("d3", DS, gsub("p3", "t3"), ["p3", "t3"])
        step("wb", DS, gsub("t2", "t0"), ["t2", "t0"])
        step("hb", DS, gsub("t3", "t1"), ["t3", "t1"])
        step("wa", DS, vsub("p2", "p0"), ["p2", "p0"])
        step("ha", DS, vsub("p3", "p1"), ["p3", "p1"])
        # ---- relus ---------------------------------------------------------
        step("r0p", DS, grelu("d0"), ["d0"])
        step("r0n", DS, grelun("d0"), ["d0"])
        step("r2p", DS, grelu("d2"), ["d2"])
        step("r2n", DS, grelun("d2"), ["d2"])
        step("r1p", DS, arelu("d1"), ["d1"])
        step("r1n", DS, arelu("d1", -1.0), ["d1"])
        step("r3p", DS, arelu("d3"), ["d3"])
        step("r3n", DS, arelu("d3", -1.0), ["d3"])
        # ---- intersection --------------------------------------------------
        step("g1", DS, vadd("r0p", "r2n"), ["r0p", "r2n"])
        step("g2", DS, vadd("r1p", "r3n"), ["r1p", "r3n"])
        step("z1", DS, vsub("wb", "g1"), ["wb", "g1"])
        step("z2", DS, vsub("hb", "g2"), ["hb", "g2"])
        step("inter", DS, cust(ops["RELU_MUL"], "z1", "z2"), ["z1", "z2"])
        # ---- enclosing box / center distance (fp32: values overflow f16) ---
        step("h1", DS, vadd("r2p", "r0n"), ["r2p", "r0n"])
        step("h2", DS, vadd("r3p", "r1n"), ["r3p", "r1n"])
        step("cwv", DS, vadd("wb", "h1"), ["wb", "h1"])
        step("chv", DS, vadd("hb", "h2"), ["hb", "h2"])
        step("cw2", DS, lambda d, e: S.activation(
            d[:], e["cwv"][:], Squ, scale=0.0625), ["cwv"])
        step("ch2", DS, lambda d, e: S.activation(
            d[:], e["chv"][:], Squ, scale=0.0625), ["chv"])
        step("diag4", DS, vadd("cw2", "ch2"), ["cw2", "ch2"])
        step("rdiag", DS, recipf("diag4"), ["diag4"])
        step("cxv", DS, vadd("d0", "d2"), ["d0", "d2"])
        step("cyv", DS, vadd("d1", "d3"), ["d1", "d3"])
        step("cx2", DS, lambda d, e: S.activation(
            d[:], e["cxv"][:], Squ, scale=0.03125), ["cxv"])
        step("cy2", DS, lambda d, e: S.activation(
            d[:], e["cyv"][:], Squ, scale=0.03125), ["cyv"])
        step("cent4", DS, vadd("cx2", "cy2"), ["cx2", "cy2"])
        step("cd", DS, vmul("cent4", "rdiag"), ["cent4", "rdiag"])
        # ---- iou -----------------------------------------------------------
        step("A1", DS, vmul("wa", "ha"), ["wa", "ha"])
        step("A2", DS, vmul("wb", "hb"), ["wb", "hb"])
        step("u12", DS, vadd("A1", "A2"), ["A1", "A2"])
        step("union", DS, vsub("u12", "inter"), ["u12", "inter"])
        step("runion", DS, recipf("union"), ["union"])
        step("iou", DS, vmul("inter", "runion"), ["inter", "runion"])
        step("diou", DS, vsub("iou", "cd"), ["iou", "cd"])
        # ---- aspect-ratio term ---------------------------------------------
        step("n1", DS, vmul("wa", "hb"), ["wa", "hb"])
        step("n2", DS, vmul("wb", "ha"), ["wb", "ha"])
        step("num", DS, vsub("n1", "n2"), ["n1", "n2"])
        step("de1", DS, vmul("ha", "hb"), ["ha", "hb"])
        step("de2", DS, vmul("wa", "wb"), ["wa", "wb"])
        step("den", DS, vadd("de1", "de2"), ["de1", "de2"])
        step("n1", DS, vmul("wa", "hb"), ["wa", "hb"])
        step("n2", DS, vmul("wb", "ha"), ["wb", "ha"])
        step("num", DS, vsub("n1", "n2"), ["n1", "n2"])
        step("de1", DS, vmul("ha", "hb"), ["ha", "hb"])
        step("de2", DS, vmul("wa", "wb"), ["wa", "wb"])
        step("den", DS, vadd("de1", "de2"), ["de1", "de2"])
        step("rden", DS, recipf("den"), ["den"])
        step("T", DS, vmul("num", "rden"), ["num", "rden"])
        step("rT", DS, recipf("T"), ["T"])
        step("arg", DS, cust(ops["ARGSEL"], "T", "rT"), ["T", "rT"])
        step("pp1", DS, cust(ops["ATAN_P1"], "arg", None,
                             s0=ATAN_C[5], s1=ATAN_C[4], imm2=ATAN_C[3]), ["arg"])
        step("pp", DS, cust(ops["ATAN_P2"], "arg", "pp1",
                            s0=ATAN_C[2], s1=ATAN_C[1], imm2=ATAN_C[0]),
             ["arg", "pp1"])
        # p is (2/pi)-scaled, so the |T|>1 branch constant is sign(T)*1
        step("dth", DS, cust(ops["RECON"], "T", "pp",
                             s0=1.0, s1=-1.0), ["T", "pp"])
        step("v", DS, lambda d, e: S.activation(
            d[:], e["dth"][:], Squ, scale=2.0 / math.pi), ["dth"])
        step("v2", F32, lambda d, e: S.activation(d[:], e["v"][:], Squ), ["v"])
        # ---- alpha*v -------------------------------------------------------
        step("om", DS, lambda d, e: V.tensor_scalar(
            out=d[:], in0=e["iou"][:], scalar1=-1.0, scalar2=1.0000001,
            op0=TT.mult, op1=TT.add), ["iou"])
        step("vmi1", DS, vadd("v", "om"), ["v", "om"])
        step("rvd", F32, recipf("vmi1"), ["vmi1"])
        step("av", DS, vmul("v2", "rvd"), ["v2", "rvd"])
        step("ciou", DS, vsub("diou", "av"), ["diou", "av"])
        # ---- weighted clipped loss + reduce --------------------------------
        step("w", DS, vmul("mk", "bn"), ["mk", "bn"])
        return steps

    with tile.TileContext(nc) as tc:
        with tc.tile_pool(name="io", bufs=2) as pio, \
             tc.tile_pool(name="tmp", bufs=2) as ptmp, \
             tc.tile_pool(name="accp", bufs=1) as pacc:
            acc_sb = pacc.tile([P, NCH], F32, tag="acc_sb", name="acc_sb")
            for k in range(NCH):
                sl = slice(k * R, (k + 1) * R)
                env = {}
                # order loads so the first compute ops' operands land first
                for nm in ("p0", "t0", "p2", "t2", "p1", "t1", "p3", "t3"):
                    t = pio.tile([P, R], F32, tag=f"in_{nm}", name=f"in_{nm}")
                    nc.sync.dma_start(out=t[:], in_=dram[nm][:, sl])
                    env[nm] = t
                tb = pio.tile([P, R], DT_SMALL, tag="in_bn", name="in_bn")
                nc.gpsimd.dma_start(out=tb[:], in_=dram["bn"][:, sl])
                env["bn"] = tb
                tm = pio.tile([P, R], DT_SMALL, tag="in_mk", name="in_mk")
                nc.gpsimd.dma_start(out=tm[:], in_=dram["mk"][:, sl])
                env["mk"] = tm

                steps = pipeline(nc, env, None, None)
                # liveness: last step index using each name
                last_use = {}
                for i, (_, _, _, ins) in enumerate(steps):
                    for nm in ins:
                        last_use[nm] = i
                # buffer free-list per dtype
                free = {}
                owner = {}

                def take(dtype):
                    lst = free.setdefault(dtype, [])
                    if lst:
                        return lst.pop()
                    idx = take.counter = getattr(take, "counter", 0) + 1
                    return ptmp.tile([P, R], dtype, tag=f"tb_{dtype}_{idx}",
                                     name=f"tb_{dtype}_{idx}")

                for i, (nm, dtype, fn, ins) in enumerate(steps):
                    dst = take(dtype)
                    owner[nm] = (dst, dtype)
                    fn(dst, env)
                    env[nm] = dst
                    for used in ins:
                        if last_use.get(used) == i and used in owner:
                            bt, bd = owner.pop(used)
                            free.setdefault(bd, []).append(bt)

                # final fused loss+mask+reduce
                dummy = ptmp.tile([P, R], DT_SMALL, tag="dummy", name="dummy")
                nc.vector._custom_dve(
                    _my_ops["ANT_LOSS_ACC"], out=dummy[:],
                    in0=env["ciou"][:], in1=env["w"][:],
                    accum_out=acc_sb[:, k:k + 1])
            nc.sync.dma_start(out=out_acc[:], in_=acc_sb[:])

    nc.compile()
    _cache["nc"] = nc
    return nc


# ------------------------------- host side ---------------------------------

def _shard_inputs(predicts_bbox, targets_bbox, valid_masks, box_norm):
    in_maps = []
    pr = np.asarray(predicts_bbox, dtype=np.float32).reshape(B, A, 4)
    tg = np.asarray(targets_bbox, dtype=np.float32).reshape(B, A, 4)
    vm = np.asarray(valid_masks)
    bn = np.asarray(box_norm, dtype=np.float32)
    for c in range(N_CORES):
        rows = slice(c * B_LOC, (c + 1) * B_LOC)
        pc = pr[rows].reshape(E, 4)
        tc_ = tg[rows].reshape(E, 4)
        m = {}
        for i in range(4):
            m[f"p{i}"] = np.ascontiguousarray(pc[:, i]).reshape(P, F)
            m[f"t{i}"] = np.ascontiguousarray(tc_[:, i]).reshape(P, F)
        m["bn"] = np.ascontiguousarray(bn[rows]).reshape(P, F)
        m["mk"] = np.ascontiguousarray(
            vm[rows]).reshape(P, F).astype(np.uint8)
        in_maps.append(m)
    return in_maps


def kernel(predicts_bbox, targets_bbox, valid_masks, box_norm, cls_norm):
    nc = _build_program()
    in_maps = _shard_inputs(predicts_bbox, targets_bbox, valid_masks, box_norm)
    res = bass_utils.run_bass_kernel_spmd(nc, in_maps, core_ids=list(range(N_CORES)))
    total = np.float64(0.0)
    for c in range(N_CORES):
        total += res.results[c]["acc"].astype(np.float64).sum()
    out = np.float32(total / np.float64(np.asarray(cls_norm)))
    return np.asarray(out, dtype=np.float32)


# revision 37
# speedup vs baseline: 1.0717x; 1.0296x over previous
"""Trainium2 Bass kernel for nn_BoxLoss (masked weighted CIoU loss).

Contract: kernel(**inputs) takes the FULL unsharded inputs
  predicts_bbox [128, 33600, 4] f32, targets_bbox [128, 33600, 4] f32,
  valid_masks [128, 33600] bool, box_norm [128, 33600] f32, cls_norm () f32
and returns the FULL scalar output, sharding batch rows across 8 NeuronCores
internally (pure data parallel, per the sharding hint).

Per-core layout: 16 batch rows x 33600 anchors = 537600 elements laid out
[128 partitions, 4200] (partition-major, each partition owns a contiguous
span). Box coords are de-interleaved on host into planar channels so every
device-side access is contiguous.

Math notes (exact reformulation of the reference):
  d_c  = p_c - t_c ;  wb = t2-t0, hb = t3-t1, wa = p2-p0, ha = p3-p1
  iw   = wb - relu(-d2) - relu(d0)       (== min(p2,t2) - max(p0,t0))
  cw   = wb + relu(d2) + relu(-d0)       (== max(p2,t2) - min(p0,t0))
  cent*4 = (d0+d2)^2 + (d1+d3)^2 ;  diag*4 = (2cw)^2 + (2ch)^2
  => cent*0.25/diag = cent4 / diag4
  atan(u)-atan(v) = atan(T), T=(wa*hb - wb*ha)/(ha*hb + wa*wb), via
  |T|<=1 ? atan(T) : sign(T)*pi/2 - atan(1/T), atan by deg-11 minimax poly.
  Non-overlapping pairs give inter=0 -> ciou = -cd-av < 0 -> loss contrib
  is exactly w (the clip), so fp16 intermediates only perturb overlapping
  pairs (small relative coords) when DT_SMALL = float16.
"""

import sys

if "/opt/trn_rl_repo" not in sys.path:
    sys.path.insert(0, "/opt/trn_rl_repo")

import math
import numpy as np

import concourse.bacc as bacc
from concourse import mybir, tile
from concourse import bass_utils
from concourse import dve_ops as dvo
from concourse.dve_spec import (
    Spec, Src0, Src1, C0, C1, C2, Zero, One, AluOp,
    relu, sq, maxx, minn, select, lower, _has_src1,
)
from concourse.dve_uop import DveOpSpec
from operator import add as _op_add

# ------------------------------- config ------------------------------------
B, A = 128, 33600
N_CORES = 8
B_LOC = B // N_CORES                # 16 batch rows per core
E = B_LOC * A                       # 537600 elements per core
P = 128                             # partitions
F = E // P                          # 4200 free elements per partition
R = 1050                            # chunk free size (divides F)
NCH = F // R

F32 = mybir.dt.float32
F16 = mybir.dt.float16
U8 = mybir.dt.uint8

# dtype of the "small" intermediate chain. float32 is the safe default;
# float16 doubles stock DVE tensor_tensor throughput.
DT_SMALL = F16

HALF_PI = math.pi / 2.0
# minimax atan(x) ~ x*(c0 + c1 z + ... + c5 z^5), z=x^2, |x|<=1, err 1.7e-6
ATAN_C = [0.9999772562021794, -0.3326237246324494, 0.19354622050707823,
          -0.11644164122245204, 0.05266424416536723, -0.011725888127135233]

# --------------------------- custom DVE ops --------------------------------
_my_ops = {}


def _register(name, spec, subdim=False):
    if name in _my_ops:
        return _my_ops[name]
    existing = {op.name: op for op in dvo.OPS}
    if name in existing:
        _my_ops[name] = existing[name]
        return existing[name]
    opcode = dvo._CUSTOM_DVE_ROW_BASE + len(dvo.OPS)
    shas = {}
    for ver in ("v3", "v4"):
        tmp = DveOpSpec(name=name, opcode=opcode, uops=lower(spec, ver=ver),
                        rd1_en=_has_src1(spec))
        shas[ver] = tmp.sha(ver)
    op = dvo.DveOp(name, spec, subdim=subdim, uops_sha=shas)
    dvo.OPS.append(op)
    dvo._SUB_OPCODE_FOR_NAME[name] = opcode
    dvo.CUSTOM_DVE_SPECS[name] = spec
    _my_ops[name] = op
    return op


def _ref_with_sum(body_fn):
    def _r(in0, in1, s0, s1, imm2):
        b = body_fn(in0, in1, s0, s1, imm2).astype(np.float32)
        return b, b.reshape(b.shape[0], -1).sum(-1, keepdims=True)
    return _r


def _registry():
    ops = {}
    ops["RELUPN"] = _register("ANT_RELUPN", Spec(
        body=relu(Src0) + relu(Zero - Src1),
        reference=lambda in0, in1, s0, s1, imm2:
            np.maximum(in0.astype(np.float32), 0)
            + np.maximum(-in1.astype(np.float32), 0),
    ))
    ops["COMB_ALPHA"] = _register("ANT_COMB_ALPHA", Spec(
        body=Src0 * C0 - Src1,
        reference=lambda in0, in1, s0, s1, imm2:
            in0.astype(np.float32) * s0 - in1.astype(np.float32),
    ))
    ops["RELU_MUL"] = _register("ANT_RELU_MUL", Spec(
        body=relu(Src0) * relu(Src1),
        reference=lambda in0, in1, s0, s1, imm2:
            np.maximum(in0.astype(np.float32), 0) * np.maximum(in1.astype(np.float32), 0),
    ))
    ops["SQ_ADD"] = _register("ANT_SQ_ADD", Spec(
        body=sq(Src0 + Src1),
        reference=lambda in0, in1, s0, s1, imm2:
            np.square(in0.astype(np.float32) + in1.astype(np.float32)),
    ))
    ops["SQ_ADD_S"] = _register("ANT_SQ_ADD_S", Spec(
        body=sq((Src0 + Src1) * C2),
        reference=lambda in0, in1, s0, s1, imm2:
            np.square((in0.astype(np.float32) + in1.astype(np.float32)) * imm2),
    ))
    ops["ARGSEL"] = _register("ANT_ARGSEL", Spec(
        body=select(sq(Src0) <= One, Src0, Src1),
        reference=lambda in0, in1, s0, s1, imm2:
            np.where(in0.astype(np.float32) ** 2 <= 1.0, in0, in1).astype(np.float32),
    ))
    _z = sq(Src0)
    ops["ATAN_P1"] = _register("ANT_ATAN_P1", Spec(
        body=(C0 * _z + C1) * _z + C2,
        reference=lambda in0, in1, s0, s1, imm2:
            ((s0 * in0.astype(np.float32) ** 2 + s1) * in0.astype(np.float32) ** 2 + imm2),
    ))
    _z2 = sq(Src0)
    ops["ATAN_P2"] = _register("ANT_ATAN_P2", Spec(
        body=(((Src1 * _z2 + C0) * _z2 + C1) * _z2 + C2) * Src0,
        reference=lambda in0, in1, s0, s1, imm2: (
            (((in1.astype(np.float32) * in0.astype(np.float32) ** 2 + s0)
              * in0.astype(np.float32) ** 2 + s1)
             * in0.astype(np.float32) ** 2 + imm2) * in0.astype(np.float32)),
    ))
    ops["RECON"] = _register("ANT_ATAN_RECON", Spec(
        body=select(sq(Src0) <= One, Src1,
                    select(Src0 >= Zero, C0, C1) - Src1),
        reference=lambda in0, in1, s0, s1, imm2: np.where(
            in0.astype(np.float32) ** 2 <= 1.0, in1,
            np.where(in0 >= 0, s0, s1) - in1).astype(np.float32),
    ))
    ops["LOSS_ACC"] = _register("ANT_LOSS_ACC", Spec(
        body=minn(relu(One - Src0), One) * Src1,
        accum=_op_add,
        reference=_ref_with_sum(
            lambda in0, in1, s0, s1, imm2:
                np.minimum(np.maximum(1.0 - in0.astype(np.float32), 0.0), 1.0)
                * in1.astype(np.float32)),
    ))
    return ops


# ------------------------------ program ------------------------------------
_cache = {}


def _build_program():
    if "nc" in _cache:
        return _cache["nc"]
    ops = _registry()
    RF = dvo.RECIPROCAL_APPROX_FAST
    RFC = dvo.RECIP_APPROX_FAST_CONSTS

    nc = bacc.Bacc("TRN2", debug=False, target_bir_lowering=False)

    def register_const_ap(dtype, value):
        tensor = nc.alloc_sbuf_tensor(f"const-{dtype.name}-{value}", [128, 1], dtype)
        nc.gpsimd.memset(tensor.ap(), value)
        nc.const_aps.aps[(dtype, value)] = tensor.ap()

    register_const_ap(F32, 1.0000001)
    nc.all_engine_barrier()
    dram = {}
    for nm in ("p0", "p1", "p2", "p3", "t0", "t1", "t2", "t3", "bn"):
        dram[nm] = nc.dram_tensor(nm, [P, F], F32, kind="ExternalInput").ap()
    dram["mk"] = nc.dram_tensor("mk", [P, F], U8, kind="ExternalInput").ap()
    out_acc = nc.dram_tensor("acc", [P, NCH], F32, kind="ExternalOutput").ap()

    DS = DT_SMALL

    # (name, dtype, engine, emit(env, dst)) — emitted in order; buffers are
    # assigned by last-use liveness below. engine: V=vector, A=act, G=gpsimd.
    def pipeline(nc, env, alloc, free_after):
        V, S, G = nc.vector, nc.scalar, nc.gpsimd
        Relu = mybir.ActivationFunctionType.Relu
        Squ = mybir.ActivationFunctionType.Square

        steps = []

        def step(name, dtype, fn, ins):
            steps.append((name, dtype, fn, ins))

        TT = mybir.AluOpType

        def vsub(a, b):
            return lambda d, e: V.tensor_sub(out=d[:], in0=e[a][:], in1=e[b][:])

        def vadd(a, b):
            return lambda d, e: V.tensor_add(out=d[:], in0=e[a][:], in1=e[b][:])

        def vmul(a, b):
            return lambda d, e: V.tensor_mul(out=d[:], in0=e[a][:], in1=e[b][:])

        def gsub(a, b):  # subtract on GPSIMD (frees DVE cycles)
            return lambda d, e: G.tensor_sub(out=d[:], in0=e[a][:], in1=e[b][:])

        def grelu(a):  # relu(x) on DVE tensor_scalar
            return lambda d, e: V.tensor_scalar(
                out=d[:], in0=e[a][:], scalar1=0.0, scalar2=None, op0=TT.max)

        def grelun(a):  # relu(-x) on DVE
            return lambda d, e: V.tensor_scalar(
                out=d[:], in0=e[a][:], scalar1=-1.0, scalar2=0.0,
                op0=TT.mult, op1=TT.max)

        def arelu(a, scale=1.0):  # relu(scale*x) on ACT
            return lambda d, e: S.activation(d[:], e[a][:], Relu, scale=scale)

        def cust(op, a, b=None, **kw):
            def _f(d, e):
                nc.vector._custom_dve(
                    op, out=d[:], in0=e[a][:],
                    in1=(e[b][:] if b is not None else None), **kw)
            return _f

        def recipf(a):
            return cust(RF, a, None, s0=RFC["s0"], s1=RFC["s1"], imm2=RFC["imm2"])

        # ---- prologue: fp32 in, DS out -------------------------------------
        step("d0", DS, gsub("p0", "t0"), ["p0", "t0"])
        step("d1", DS, gsub("p1", "t1"), ["p1", "t1"])
        step("d2", DS, gsub("p2", "t2"), ["p2", "t2"])
        step("d3", DS, gsub("p3", "t3"), ["p3", "t3"])
        step("wb", DS, gsub("t2", "t0"), ["t2", "t0"])
        step("hb", DS, gsub("t3", "t1"), ["t3", "t1"])
        step("wa", DS, vsub("p2", "p0"), ["p2", "p0"])
        step("ha", DS, vsub("p3", "p1"), ["p3", "p1"])
        # ---- relus ---------------------------------------------------------
        step("r0p", DS, grelu("d0"), ["d0"])
        step("r0n", DS, grelun("d0"), ["d0"])
        step("r2p", DS, grelu("d2"), ["d2"])
        step("r2n", DS, grelun("d2"), ["d2"])
        step("r1p", DS, arelu("d1"), ["d1"])
        step("r1n", DS, arelu("d1", -1.0), ["d1"])
        step("r3p", DS, arelu("d3"), ["d3"])
        step("r3n", DS, arelu("d3", -1.0), ["d3"])
        # ---- intersection --------------------------------------------------
        step("g1", DS, vadd("r0p", "r2n"), ["r0p", "r2n"])
        step("g2", DS, vadd("r1p", "r3n"), ["r1p", "r3n"])
        step("z1", DS, vsub("wb", "g1"), ["wb", "g1"])
        step("z2", DS, vsub("hb", "g2"), ["hb", "g2"])
        step("inter", DS, cust(ops["RELU_MUL"], "z1", "z2"), ["z1", "z2"])
        # ---- enclosing box / center distance (fp32: values overflow f16) ---
        step("h1", DS, vadd("r2p", "r0n"), ["r2p", "r0n"])
        step("h2", DS, vadd("r3p", "r1n"), ["r3p", "r1n"])
        step("cwv", DS, vadd("wb", "h1"), ["wb", "h1"])
        step("chv", DS, vadd("hb", "h2"), ["hb", "h2"])
        step("cw2", DS, lambda d, e: S.activation(
            d[:], e["cwv"][:], Squ, scale=0.0625), ["cwv"])
        step("ch2", DS, lambda d, e: S.activation(
            d[:], e["chv"][:], Squ, scale=0.0625), ["chv"])
        step("diag4", DS, vadd("cw2", "ch2"), ["cw2", "ch2"])
        step("rdiag", DS, recipf("diag4"), ["diag4"])
        step("cxv", DS, vadd("d0", "d2"), ["d0", "d2"])
        step("cyv", DS, vadd("d1", "d3"), ["d1", "d3"])
        step("cx2", DS, lambda d, e: S.activation(
            d[:], e["cxv"][:], Squ, scale=0.03125), ["cxv"])
        step("cy2", DS, lambda d, e: S.activation(
            d[:], e["cyv"][:], Squ, scale=0.03125), ["cyv"])
        step("cent4", DS, vadd("cx2", "cy2"), ["cx2", "cy2"])
        step("cd", DS, vmul("cent4", "rdiag"), ["cent4", "rdiag"])
        # ---- iou -----------------------------------------------------------
        step("A1", DS, vmul("wa", "ha"), ["wa", "ha"])
        step("A2", DS, vmul("wb", "hb"), ["wb", "hb"])
        step("u12", DS, vadd("A1", "A2"), ["A1", "A2"])
        step("union", DS, vsub("u12", "inter"), ["u12", "inter"])
        step("runion", DS, recipf("union"), ["union"])
        step("iou", DS, vmul("inter", "runion"), ["inter", "runion"])
        step("diou", DS, vsub("iou", "cd"), ["iou", "cd"])
        # ---- aspect-ratio term ---------------------------------------------
        step("n1", DS, vmul("wa", "hb"), ["wa", "hb"])
        step("n2", DS, vmul("wb", "ha"), ["wb", "ha"])
        step("num", DS, vsub("n1", "n2"), ["n1", "n2"])
        step("de1", DS, vmul("ha", "hb"), ["ha", "hb"])
        step("de2", DS, vmul("wa", "wb"), ["wa", "wb"])
        step("den", DS, vadd("de1", "de2"), ["de1", "de2"])
        step("n1", DS, vmul("wa", "hb"), ["wa", "hb"])
        step("n2", DS, vmul("wb", "ha"), ["wb", "ha"])
        step("num", DS, vsub("n1", "n2"), ["n1", "n2"])
        step("de1", DS, vmul("ha", "hb"), ["ha", "hb"])
        step("de2", DS, vmul("wa", "wb"), ["wa", "wb"])
        step("den", DS, vadd("de1", "de2"), ["de1", "de2"])
        step("rden", DS, recipf("den"), ["den"])
        step("T", DS, vmul("num", "rden"), ["num", "rden"])
        step("rT", DS, recipf("T"), ["T"])
        step("arg", DS, cust(ops["ARGSEL"], "T", "rT"), ["T", "rT"])
        step("pp1", DS, cust(ops["ATAN_P1"], "arg", None,
                             s0=ATAN_C[5], s1=ATAN_C[4], imm2=ATAN_C[3]), ["arg"])
        step("pp", DS, cust(ops["ATAN_P2"], "arg", "pp1",
                            s0=ATAN_C[2], s1=ATAN_C[1], imm2=ATAN_C[0]),
             ["arg", "pp1"])
        # p is (2/pi)-scaled, so the |T|>1 branch constant is sign(T)*1
        step("dth", DS, cust(ops["RECON"], "T", "pp",
                             s0=1.0, s1=-1.0), ["T", "pp"])
        step("v", DS, lambda d, e: S.activation(
            d[:], e["dth"][:], Squ, scale=2.0 / math.pi), ["dth"])
        step("v2", F32, lambda d, e: S.activation(d[:], e["v"][:], Squ), ["v"])
        # ---- alpha*v -------------------------------------------------------
        step("om", DS, lambda d, e: V.tensor_scalar(
            out=d[:], in0=e["iou"][:], scalar1=-1.0, scalar2=1.0000001,
            op0=TT.mult, op1=TT.add), ["iou"])
        step("vmi1", DS, vadd("v", "om"), ["v", "om"])
        step("rvd", F32, recipf("vmi1"), ["vmi1"])
        step("av", DS, vmul("v2", "rvd"), ["v2", "rvd"])
        step("ciou", DS, vsub("diou", "av"), ["diou", "av"])
        # ---- weighted clipped loss + reduce --------------------------------
        step("w", DS, vmul("mk", "bn"), ["mk", "bn"])
        return steps

    with tile.TileContext(nc) as tc:
        with tc.tile_pool(name="io", bufs=2) as pio, \
             tc.tile_pool(name="tmp", bufs=2) as ptmp, \
             tc.tile_pool(name="accp", bufs=1) as pacc:
            acc_sb = pacc.tile([P, NCH], F32, tag="acc_sb", name="acc_sb")
            for k in range(NCH):
                sl = slice(k * R, (k + 1) * R)
                env = {}
                # order loads so the first compute ops' operands land first
                for nm in ("p0", "t0", "p2", "t2", "p1", "t1", "p3", "t3"):
                    t = pio.tile([P, R], F32, tag=f"in_{nm}", name=f"in_{nm}")
                    nc.sync.dma_start(out=t[:], in_=dram[nm][:, sl])
                    env[nm] = t
                tb = pio.tile([P, R], DT_SMALL, tag="in_bn", name="in_bn")
                nc.gpsimd.dma_start(out=tb[:], in_=dram["bn"][:, sl])
                env["bn"] = tb
                tm = pio.tile([P, R], DT_SMALL, tag="in_mk", name="in_mk")
                nc.gpsimd.dma_start(out=tm[:], in_=dram["mk"][:, sl])
                env["mk"] = tm

                steps = pipeline(nc, env, None, None)
                # liveness: last step index using each name
                last_use = {}
                for i, (_, _, _, ins) in enumerate(steps):
                    for nm in ins:
                        last_use[nm] = i
                # buffer free-list per dtype
                free = {}
                owner = {}

                def take(dtype):
                    lst = free.setdefault(dtype, [])
                    if lst:
                        return lst.pop()
                    idx = take.counter = getattr(take, "counter", 0) + 1
                    return ptmp.tile([P, R], dtype, tag=f"tb_{dtype}_{idx}",
                                     name=f"tb_{dtype}_{idx}")

                for i, (nm, dtype, fn, ins) in enumerate(steps):
                    dst = take(dtype)
                    owner[nm] = (dst, dtype)
                    fn(dst, env)
                    env[nm] = dst
                    for used in ins:
                        if last_use.get(used) == i and used in owner:
                            bt, bd = owner.pop(used)
                            free.setdefault(bd, []).append(bt)

                # final fused loss+mask+reduce; reuse a dead f16 buffer
                fl = free.get(DT_SMALL) or []
                dummy = fl[0] if fl else ptmp.tile(
                    [P, R], DT_SMALL, tag="dummy", name="dummy")
                nc.vector._custom_dve(
                    _my_ops["ANT_LOSS_ACC"], out=dummy[:],
                    in0=env["ciou"][:], in1=env["w"][:],
                    accum_out=acc_sb[:, k:k + 1])
            nc.sync.dma_start(out=out_acc[:], in_=acc_sb[:])

    nc.compile()
    _cache["nc"] = nc
    return nc


# ------------------------------- host side ---------------------------------

def _shard_inputs(predicts_bbox, targets_bbox, valid_masks, box_norm):
    in_maps = []
    pr = np.asarray(predicts_bbox, dtype=np.float32).reshape(B, A, 4)
    tg = np.asarray(targets_bbox, dtype=np.float32).reshape(B, A, 4)
    vm = np.asarray(valid_masks)
    bn = np.asarray(box_norm, dtype=np.float32)
    for c in range(N_CORES):
        rows = slice(c * B_LOC, (c + 1) * B_LOC)
        pc = pr[rows].reshape(E, 4)
        tc_ = tg[rows].reshape(E, 4)
        m = {}
        for i in range(4):
            m[f"p{i}"] = np.ascontiguousarray(pc[:, i]).reshape(P, F)
            m[f"t{i}"] = np.ascontiguousarray(tc_[:, i]).reshape(P, F)
        m["bn"] = np.ascontiguousarray(bn[rows]).reshape(P, F)
        m["mk"] = np.ascontiguousarray(
            vm[rows]).reshape(P, F).astype(np.uint8)
        in_maps.append(m)
    return in_maps


def kernel(predicts_bbox, targets_bbox, valid_masks, box_norm, cls_norm):
    nc = _build_program()
    in_maps = _shard_inputs(predicts_bbox, targets_bbox, valid_masks, box_norm)
    res = bass_utils.run_bass_kernel_spmd(nc, in_maps, core_ids=list(range(N_CORES)))
    total = np.float64(0.0)
    for c in range(N_CORES):
        total += res.results[c]["acc"].astype(np.float64).sum()
    out = np.float32(total / np.float64(np.asarray(cls_norm)))
    return np.asarray(out, dtype=np.float32)
